# revision 10
# baseline (speedup 1.0000x reference)
"""Single-head causal attention forward on 8 TRN2 NeuronCores.

Problem: x [8, 2048, 1024] f32, Wq/Wk/Wv [128, 1024] f32.
  q/k/v = x @ W.T ; S = q k^T / sqrt(128) causal ; out = softmax(S) v.

Sharding: data-parallel, one batch element per core (8 cores).
Inside each core a flash-style blocked attention over 512-token chunks:
  - host pre-transposes x[b] -> xT [1024, 2048] so the contraction dim (c)
    lands on SBUF partitions with fully-contiguous DMA lines.
  - qT/kT [h=128, t] via W-stationary matmuls; V natural [t, h] via
    x-stationary matmuls; a ones-column appended to V makes the PV matmul
    also produce the softmax denominators (column sums of exp(S^T)).
  - S^T[j, q] tiles: softmax normalization along j happens via the ones
    trick; exp runs on ScalarE with the 1/sqrt(128) scale folded into the
    activation; only the 16 diagonal 128x128 sub-blocks need a triangular
    0/1 mask; strictly-masked blocks are never computed (causal skipping).
"""

import os
import sys

for _p in ("/opt/trn_rl_repo",):
    if _p not in sys.path and os.path.isdir(_p):
        sys.path.append(_p)

import numpy as np

B, T, D, H = 8, 2048, 1024, 128
CH = 512          # token chunk (free dim of S^T tiles)
NCH = T // CH     # 4 chunks
CC = D // 128     # 8 contraction sub-tiles
NT = T // 128     # 16 token tiles
SCALE = 1.0 / np.sqrt(np.float32(H))

# matmul input dtype: "float32" (2 cyc/col), "float32r" (1.5 cyc/col,
# ~tf32 precision), "bfloat16" (1 cyc/col, halves DMA too)
MM_DT = os.environ.get("KERNEL_MM_DT", "float32r")

_CACHE = {}


def _build():
    import concourse.bacc as bacc
    import concourse.mybir as mybir
    import concourse.tile as tile

    dt = mybir.dt
    # storage dtype of tensors fed to matmuls. float32r is fp32 storage with
    # relaxed-precision matmul; the BIR verifier requires every matmul input
    # to be produced AS float32r, so tiles and dram params use it directly.
    st_dt = getattr(dt, MM_DT)

    def mm_ap(ap):
        return ap

    nc = bacc.Bacc(None)
    xT = nc.declare_dram_parameter("xT", [D, T], st_dt, isOutput=False)
    wqT = nc.declare_dram_parameter("wqT", [D, H], st_dt, isOutput=False)
    wkT = nc.declare_dram_parameter("wkT", [D, H], st_dt, isOutput=False)
    wvT = nc.declare_dram_parameter("wvT", [D, H], st_dt, isOutput=False)
    tri = nc.declare_dram_parameter("tri", [128, 128], st_dt, isOutput=False)
    ones = nc.declare_dram_parameter("ones", [128, NT * 2], st_dt, isOutput=False)
    out = nc.declare_dram_parameter("out", [T, H], dt.float32, isOutput=True)

    xT_r = xT.rearrange("(cc p) t -> p cc t", p=128)    # [128, CC, T]
    wqT_r = wqT.rearrange("(cc p) h -> p cc h", p=128)  # [128, CC, H]
    wkT_r = wkT.rearrange("(cc p) h -> p cc h", p=128)
    wvT_r = wvT.rearrange("(cc p) h -> p cc h", p=128)

    with tile.TileContext(nc) as tc:
        with (
            tc.tile_pool(name="singles", bufs=1) as singles,
            tc.tile_pool(name="xp", bufs=2) as xp,
            tc.tile_pool(name="qtp", bufs=2) as qtp,
            tc.tile_pool(name="ktp", bufs=4) as ktp,
            tc.tile_pool(name="ptp", bufs=16) as ptp,
            tc.tile_pool(name="outp", bufs=4) as outp,
            tc.tile_pool(name="recp", bufs=4) as recp,
            tc.tile_pool(name="psq", bufs=2, space="PSUM") as psq,
            tc.tile_pool(name="psv", bufs=2, space="PSUM") as psv,
            tc.tile_pool(name="pss", bufs=2, space="PSUM") as pss,
            tc.tile_pool(name="pso", bufs=2, space="PSUM") as pso,
        ):
            # --- constants / weights (loaded once) ---
            wq_sb = singles.tile([128, CC, H], st_dt)
            wk_sb = singles.tile([128, CC, H], st_dt)
            wv_sb = singles.tile([128, CC, H], st_dt)
            tri_sb = singles.tile([128, 128], st_dt)
            nc.sync.dma_start(out=wq_sb[:], in_=wqT_r[:])
            nc.sync.dma_start(out=wk_sb[:], in_=wkT_r[:])
            nc.sync.dma_start(out=wv_sb[:], in_=wvT_r[:])
            nc.sync.dma_start(out=tri_sb[:], in_=tri[:])
            # V' = [V | 1]; ones column written once (DMA: memset doesn't
            # support float32r)
            v_sb = singles.tile([128, NT, H + 4], st_dt)
            nc.sync.dma_start(
                out=v_sb[:, :, H : H + 2], in_=ones.rearrange("p (n o) -> p n o", o=2)
            )

            kt_tiles = []
            for qc in range(NCH):
                q0 = qc * CH
                # --- load xT chunk [128, CC, CH] ---
                xt = xp.tile([128, CC, CH], st_dt)
                nc.sync.dma_start(out=xt[:], in_=xT_r[:, :, q0 : q0 + CH])

                # --- qT, kT for this chunk: [h=128, CH] ---
                qps = psq.tile([128, CH], dt.float32, tag="qk")
                for cc in range(CC):
                    nc.tensor.matmul(
                        qps[:],
                        mm_ap(wq_sb[:, cc, :]),
                        mm_ap(xt[:, cc, :]),
                        start=(cc == 0),
                        stop=(cc == CC - 1),
                    )
                qt = qtp.tile([128, CH], st_dt)
                nc.vector.tensor_copy(qt[:], qps[:])

                kps = psq.tile([128, CH], dt.float32, tag="qk")
                for cc in range(CC):
                    nc.tensor.matmul(
                        kps[:],
                        mm_ap(wk_sb[:, cc, :]),
                        mm_ap(xt[:, cc, :]),
                        start=(cc == 0),
                        stop=(cc == CC - 1),
                    )
                kt = ktp.tile([128, CH], st_dt)
                nc.vector.tensor_copy(kt[:], kps[:])
                kt_tiles.append(kt)

                # --- V natural [t, h] for the 4 token tiles of the chunk ---
                for ti in range(4):
                    jt = qc * 4 + ti
                    vps = psv.tile([128, H], dt.float32)
                    for cc in range(CC):
                        nc.tensor.matmul(
                            vps[:],
                            mm_ap(xt[:, cc, ti * 128 : (ti + 1) * 128]),
                            mm_ap(wv_sb[:, cc, :]),
                            start=(cc == 0),
                            stop=(cc == CC - 1),
                        )
                    nc.vector.tensor_copy(v_sb[:, jt, 0:H], vps[:])

                # --- S^T tiles + exp for this q-chunk ---
                pts = []
                for jt in range(qc * 4 + 4):
                    sps = pss.tile([128, CH], dt.float32)
                    kt_src = kt_tiles[jt // 4]
                    nc.tensor.matmul(
                        sps[:],
                        mm_ap(kt_src[:, (jt % 4) * 128 : (jt % 4 + 1) * 128]),
                        mm_ap(qt[:]),
                        start=True,
                        stop=True,
                    )
                    pt = ptp.tile([128, CH], st_dt)
                    if jt < qc * 4:
                        # fully-valid block
                        nc.scalar.activation(
                            pt[:], sps[:], mybir.ActivationFunctionType.Exp,
                            scale=float(SCALE),
                        )
                    else:
                        # diagonal-crossing block: columns < vstart are never
                        # read downstream; the [vstart, vstart+128) sub-block
                        # needs the triangular mask
                        vstart = (jt - qc * 4) * 128
                        nc.scalar.activation(
                            pt[:, vstart:CH], sps[:, vstart:CH],
                            mybir.ActivationFunctionType.Exp,
                            scale=float(SCALE),
                        )
                        nc.vector.tensor_mul(
                            pt[:, vstart : vstart + 128],
                            pt[:, vstart : vstart + 128],
                            tri_sb[:],
                        )
                    pts.append(pt)

                # --- PV + normalize + store, per 128-token q tile ---
                for ti in range(4):
                    qi = qc * 4 + ti
                    ops = pso.tile([128, H + 4], dt.float32)
                    for jt in range(qi + 1):
                        nc.tensor.matmul(
                            ops[:, 0 : H + 2],
                            mm_ap(pts[jt][:, ti * 128 : (ti + 1) * 128]),
                            mm_ap(v_sb[:, jt, 0 : H + 2]),
                            start=(jt == 0),
                            stop=(jt == qi),
                        )
                    rec = recp.tile([128, 1], dt.float32)
                    nc.vector.reciprocal(rec[:], ops[:, H : H + 1])
                    ob = outp.tile([128, H], dt.float32)
                    nc.vector.tensor_scalar_mul(ob[:], ops[:, 0:H], rec[:])
                    nc.sync.dma_start(
                        out=out[qi * 128 : (qi + 1) * 128, :], in_=ob[:]
                    )

    nc.compile()
    return nc


def _get_nc():
    if "nc" not in _CACHE:
        _CACHE["nc"] = _build()
    return _CACHE["nc"]


def _host_dtype():
    if MM_DT == "bfloat16":
        import ml_dtypes

        return ml_dtypes.bfloat16
    return np.float32


def _in_maps(x, Wq, Wk, Wv):
    hdt = _host_dtype()
    wq = np.ascontiguousarray(np.asarray(Wq, dtype=np.float32).T).astype(hdt)
    wk = np.ascontiguousarray(np.asarray(Wk, dtype=np.float32).T).astype(hdt)
    wv = np.ascontiguousarray(np.asarray(Wv, dtype=np.float32).T).astype(hdt)
    tri = np.triu(np.ones((128, 128), dtype=np.float32)).astype(hdt)
    ones = np.ones((128, NT * 2), dtype=np.float32).astype(hdt)
    x = np.asarray(x, dtype=np.float32)
    maps = []
    for b in range(B):
        xt = np.ascontiguousarray(x[b].T).astype(hdt)
        maps.append(
            {"xT": xt, "wqT": wq, "wkT": wk, "wvT": wv, "tri": tri, "ones": ones}
        )
    return maps


def kernel(x, Wq, Wk, Wv):
    from concourse.bass_utils import run_bass_kernel_spmd

    nc = _get_nc()
    res = run_bass_kernel_spmd(nc, _in_maps(x, Wq, Wk, Wv), core_ids=list(range(B)))
    return np.stack([res.results[b]["out"] for b in range(B)]).astype(np.float32)
